# revision 34
# baseline (speedup 1.0000x reference)
"""Multi-head causal attention on 8 Trainium2 NeuronCores.

Sharding: tensor-parallel over heads (2 heads/core). Each core computes QKV
projection + attention for its 2 heads over all 4096 tokens, then a PARTIAL
output projection against its 128 rows of Wo (bf16). The all-reduce over the
8 partial outputs happens on the host — no on-device collective at all, so
cores are fully decoupled (launch skew and per-core clock-throttle skew no
longer serialize through a collective barrier).

Layout strategy (per core):
  - everything bf16 on device (host pre-casts x/weights); PSUM accumulation
    stays fp32 so only input quantization error is added (~0.4%).
  - qkvT = W^T @ x^T computed in transposed [feature, token] layout
    (x^T prepared on host) so attention contractions need no transposes of
    Q/K.
  - scores^T[k, q] = K_tile^T.T @ Q^T with the two heads row-tiled on the PE
    (d=64 contraction each, partitions 0-63 / 64-127, concurrent).
  - softmax: exp on ScalarE straight out of PSUM with the 1/sqrt(D) scale
    folded into the activation's free affine; no max-subtraction (scores are
    O(6) so exp is safe); causal masking is a multiplicative 0/1 bf16 mask on
    the diagonal tiles only, with the column extent restricted to the
    not-fully-masked range [128*o, 512).
  - AV: av^T[d, q] accumulated over k-tiles in PSUM; lhsT = [V_k | 1] needs V
    in natural [token, d] layout, produced by PE transpose. The ones column
    gives the softmax denominator on psum row 64 for free.
  - normalize: reciprocal_approx_fast on the denominator row (single DVE op,
    ~5x faster than iterative reciprocal), broadcast across partitions by
    GpSimd partition_broadcast (idle engine; no DRAM bounce), one fused DVE
    multiply writes the bf16 attention tile.
  - output projection per (batch, q-chunk) right after that chunk's attention
    so the PE stream stays dense; partial out rows DMA'd as bf16.
"""

import os

import numpy as np
import ml_dtypes

import concourse.bass as bass
import concourse.mybir as mybir
import concourse.tile as tile
from concourse.bass_utils import run_bass_kernel_spmd
from concourse.masks import make_identity
from concourse.vector_clock import ScopedClock

F32 = mybir.dt.float32
BF16 = mybir.dt.bfloat16
AF = mybir.ActivationFunctionType


def _install_cache_nonce_hook():
    """The libneuronxla NEFF cache hashes the HLO but the BIR rides in
    backend_config (excluded from the hash), so edited kernels with the same
    I/O signature can silently hit a stale cached NEFF. Inject a hash of the
    BIR into mhlo.frontend_attributes — which IS part of the model hash —
    the same way bass2jax ships the DVE tables."""
    import hashlib
    import concourse.bass2jax as bass2jax
    from jax.interpreters import mlir

    if getattr(bass2jax, "_ant_cache_nonce_hooked", False):
        return
    bass2jax._ant_cache_nonce_hooked = True
    orig = bass2jax._accumulate_module_dve_attrs

    def patched(ctx, nc):
        orig(ctx, nc)
        op = ctx.module_context.module.operation
        cur = (
            op.attributes["mhlo.frontend_attributes"]
            if "mhlo.frontend_attributes" in op.attributes
            else None
        )
        existing = (
            {a.name: mlir.ir.StringAttr(a.attr).value for a in cur}
            if cur is not None
            else {}
        )
        existing["ant.cache_nonce"] = hashlib.sha256(
            nc.to_json_bytes()
        ).hexdigest()
        op.attributes["mhlo.frontend_attributes"] = mlir.ir.DictAttr.get(
            {k: mlir.ir.StringAttr.get(v) for k, v in existing.items()}
        )

    bass2jax._accumulate_module_dve_attrs = patched


_install_cache_nonce_hook()


B, S, DM = 2, 2048, 1024
H, D = 16, 64
NCORES = 8
HP = H // NCORES          # heads per core
T = B * S                 # 4096 tokens
NCH = T // 512            # 8 token chunks of 512
KT_PER_S = S // 128       # 16 k-tiles per sequence
QT_PER_S = S // 512       # 4 q-tiles per sequence
SCALE = 1.0 / np.sqrt(D)

# env-tunable dev flags (defaults are the shipping config)
PBCAST = os.environ.get("ATT_PBCAST", "1") == "1"

MAX_WAITS = 1  # walrus in this container rejects >1 sem-wait per instruction


def _split_waits(nc, limit=MAX_WAITS):
    """Post-pass: move excess sem-waits onto preceding same-engine nops.

    Engines dispatch in program order and a sem-wait stalls the engine's NX
    before anything later is enqueued, so carrying the waits on nops placed
    immediately before the instruction is semantically identical.
    """
    n_id = 0
    for bb in nc.main_func.blocks:
        new = []
        for inst in bb.instructions:
            si = getattr(inst, "sync_info", None)
            if si is not None and len(si.on_wait) > limit:
                waits = list(si.on_wait)
                for i in range(0, len(waits) - limit, limit):
                    nop = mybir.InstNoOp(
                        name=f"wsplit-{n_id}", ins=[], outs=[], engine=inst.engine
                    )
                    n_id += 1
                    nop.sync_info = mybir.SyncInfo(
                        on_wait=waits[i : i + limit], on_update=[]
                    )
                    new.append(nop)
                kept = waits[len(waits) - limit :]
                inst.sync_info = mybir.SyncInfo(
                    on_wait=kept, on_update=list(si.on_update)
                )
            new.append(inst)
        bb.instructions = new


class _TileCtx(tile.TileContext):
    """Work around a walrus codegen limit: the stock tail drain carries one
    sem-wait per (engine, DMA-lane), but this compiler build rejects >1-2
    waits on a Drain ("Too many sync wait commands"). Put each wait on its
    own SP nop between the drain and the final barrier instead."""

    def _drain_and_barrier(self, tick_clock, wait_clock):
        nc = self.nc
        drain_inst = nc.sync.drain()
        wait_clock.add_sem_waits(
            drain_inst.ins, ScopedClock({None: tick_clock.global_clock})
        )
        si = drain_inst.ins.sync_info
        if si is not None and len(si.on_wait) > 1:
            waits = list(si.on_wait)
            drain_inst.ins.sync_info = mybir.SyncInfo(
                on_wait=[waits[0]], on_update=list(si.on_update)
            )
            for w in waits[1:]:
                nop = nc.sync.nop(nofuse=True, hint="tail_drain_wait_split")
                nop.ins.sync_info = mybir.SyncInfo(on_wait=[w], on_update=[])

        nc.all_engine_barrier()
        assert self.sems is not None
        popped = nc._tile_sem_poison_stack.pop()
        assert popped is self._sem_poison
        nc.clear_and_free_semaphores(list(self.sems.allocated().values()))
        nc.all_engine_barrier()


def _nkt(qt, mode):
    """Number of k-tiles attended by q-tile qt (within one sequence)."""
    return 4 * (qt + 1) if mode == "causal" else KT_PER_S


def build(mode, n_mask_tiles, skip_phase3=False):
    """Build the SPMD Bass program. mode: 'causal' | 'full' | 'general'."""
    nc = bass.Bass()

    xT = nc.dram_tensor("xT", [DM, T], BF16, kind="ExternalInput")
    wq = nc.dram_tensor("wq", [DM, 128], BF16, kind="ExternalInput")
    wk = nc.dram_tensor("wk", [DM, 128], BF16, kind="ExternalInput")
    wv = nc.dram_tensor("wv", [DM, 128], BF16, kind="ExternalInput")
    wo = nc.dram_tensor("wo", [128, DM], BF16, kind="ExternalInput")
    if n_mask_tiles:
        mt = nc.dram_tensor(
            "mt", [n_mask_tiles, 128, 512], BF16, kind="ExternalInput"
        )
    out = nc.dram_tensor("out", [T, DM], BF16, kind="ExternalOutput")

    with _TileCtx(nc) as tc:
        with (
            tc.tile_pool(name="const", bufs=1) as const,
            tc.tile_pool(name="xin", bufs=3) as xin,
            tc.tile_pool(name="stage", bufs=3) as stage,
            tc.tile_pool(name="pp", bufs=6) as pp,
            tc.tile_pool(name="misc", bufs=4) as misc,
            tc.tile_pool(name="psav", bufs=2, space="PSUM") as psav,
            tc.tile_pool(name="ps512", bufs=2, space="PSUM") as ps512,
            tc.tile_pool(name="pss", bufs=4, space="PSUM") as pss,
            tc.tile_pool(name="dram", bufs=1, space="DRAM") as dram,
        ):
            # ---- resident SBUF tensors ----
            wq_sb = const.tile([128, 8, 128], BF16)
            wk_sb = const.tile([128, 8, 128], BF16)
            wv_sb = const.tile([128, 8, 128], BF16)
            nc.sync.dma_start(wq_sb[:], wq.rearrange("(o p) e -> p o e", p=128))
            nc.sync.dma_start(wk_sb[:], wk.rearrange("(o p) e -> p o e", p=128))
            nc.sync.dma_start(wv_sb[:], wv.rearrange("(o p) e -> p o e", p=128))
            # wo/mask DMAs are emitted later (after the first x chunk) so
            # they don't delay the first QKV matmuls.
            wo_sb = const.tile([128, DM], BF16)
            if n_mask_tiles:
                mt_sb = const.tile([128, n_mask_tiles, 512], BF16)

            qT_sb = const.tile([128, NCH, 512], BF16)
            kT_sb = const.tile([128, NCH, 512], BF16)
            # V in [token, feature] layout, per k-tile, per head:
            # [p=token%128, ktile, head, 80] where cols 0:64 = v, col 64 = 1.0
            v_sb = const.tile([128, T // 128, HP, 80], BF16)
            nc.vector.memset(v_sb[:, :, :, 64:65], 1.0)
            ident = const.tile([128, 128], BF16)
            make_identity(nc, ident[:])

            den_dram = dram.tile([NCH, HP, 1, 512], F32)
            rec_dram = dram.tile([NCH, HP, 1, 512], BF16)

            # ---- software-pipelining machinery: filler work (qkv pieces,
            # epilogue finishers, output projections) is queued and drained
            # one piece per attention k-tile, so every engine's strict-FIFO
            # program order interleaves filler work INTO the attention
            # stream instead of batching it at chunk boundaries.
            from collections import deque

            fillers = deque()

            def drain(n=1):
                for _ in range(n):
                    if fillers:
                        fillers.popleft()()

            def _proj(c, xt, w_sb, name):
                psum = ps512.tile(
                    [128, 512], F32, tag="ps512", name=f"ps_{name}{c}"
                )
                for kt in range(8):
                    nc.tensor.matmul(
                        psum[:],
                        w_sb[:, kt, :],
                        xt[:, kt, :],
                        start=(kt == 0),
                        stop=(kt == 7),
                    )
                return psum

            def qkv_pieces(c, strip=False):
                # the x-chunk DMA is issued NOW (prefetch at push time) so
                # the filler pieces only contain compute — a filler matmul
                # waiting on a just-issued DMA would stall the PE FIFO
                # mid-attention. strip=True splits the load per k-strip so
                # the first projection matmul starts after 128KB, not 1MB
                # (used for chunk 0 at kernel startup).
                xt = xin.tile([128, 8, 512], BF16, tag="xt", name=f"xt{c}")
                if strip:
                    for o in range(8):
                        nc.sync.dma_start(
                            xt[:, o, :],
                            xT[128 * o : 128 * (o + 1), 512 * c : 512 * (c + 1)],
                        )
                else:
                    nc.sync.dma_start(
                        xt[:],
                        xT[:, 512 * c : 512 * (c + 1)].rearrange(
                            "(o p) s -> p o s", p=128
                        ),
                    )

                def p_q():
                    nc.vector.tensor_copy(
                        qT_sb[:, c, :], _proj(c, xt, wq_sb, "q")[:]
                    )

                def p_k():
                    nc.vector.tensor_copy(
                        kT_sb[:, c, :], _proj(c, xt, wk_sb, "k")[:]
                    )

                def p_v():
                    psum = _proj(c, xt, wv_sb, "v")
                    # PE-based transpose of V into [token, feature] layout
                    vstg = stage.tile([128, 512], BF16, tag="vstg")
                    nc.vector.tensor_copy(vstg[:], psum[:])
                    ps_t = ps512.tile(
                        [128, 512], BF16, tag="ps512", name=f"ps_t{c}"
                    )
                    for sub in range(4):
                        nc.tensor.transpose(
                            ps_t[:, 128 * sub : 128 * (sub + 1)],
                            vstg[:, 128 * sub : 128 * (sub + 1)],
                            ident[:],
                        )
                    for sub in range(4):
                        ktile = 4 * c + sub
                        for h in range(HP):
                            nc.vector.tensor_copy(
                                v_sb[:, ktile, h, 0:64],
                                ps_t[
                                    :,
                                    128 * sub + 64 * h : 128 * sub
                                    + 64 * (h + 1),
                                ],
                            )

                return [p_q, p_k, p_v]

            def attention(b, qt, attnT=None):
                ch = b * QT_PER_S + qt
                nkt = _nkt(qt, mode)

                # diagonal-tile bookkeeping: mask index + valid column start
                def mask_index(kt):
                    if mode == "causal":
                        off = kt - 4 * qt
                        return off if 0 <= off < 4 else None
                    if mode == "general":
                        return qt * KT_PER_S + kt
                    return None

                def col0(kt):
                    # first not-fully-masked q column of this k-tile
                    if mode == "causal":
                        off = kt - 4 * qt
                        if 0 <= off < 4:
                            return 128 * off
                    return 0

                av = [
                    psav.tile([128, 512], F32, tag="av", name=f"av{ch}_{h}")
                    for h in range(HP)
                ]
                # per-k-tile singles: a 4-deep [128,512] scores-PSUM ring,
                # with the scores+exp emission running LEAD k-tiles ahead of
                # the AV matmuls so the PE (strict program order) never
                # reaches an AV before its exp has had time to complete.
                # Scores, exp, mask and AV are all restricted to the
                # not-fully-masked column range [c0, 512).
                LEAD = 2
                srcs_q = {}

                def emit_scores(kt):
                    c, ks = b * QT_PER_S + kt // 4, kt % 4
                    mi = mask_index(kt)
                    c0 = col0(kt)
                    srcs = []
                    for h in range(HP):
                        ps_s = pss.tile(
                            [128, 512], F32, tag="pss", name=f"s{ch}_{kt}_{h}"
                        )
                        nc.tensor.matmul(
                            ps_s[:, c0:512],
                            kT_sb[
                                64 * h : 64 * (h + 1),
                                c,
                                128 * ks : 128 * (ks + 1),
                            ],
                            qT_sb[64 * h : 64 * (h + 1), ch, c0:512],
                            start=True,
                            stop=True,
                        )
                        pt = pp.tile([128, 512], BF16, tag="p", bufs=8)
                        nc.scalar.activation(
                            pt[:, c0:512],
                            ps_s[:, c0:512],
                            AF.Exp,
                            scale=float(SCALE),
                        )
                        if mi is None:
                            srcs.append(pt[:, c0:512])
                        else:
                            pm = pp.tile([128, 512], BF16, tag="pm", bufs=6)
                            nc.vector.tensor_tensor(
                                pm[:, c0:512],
                                pt[:, c0:512],
                                mt_sb[:, mi, c0:512],
                                mybir.AluOpType.mult,
                            )
                            srcs.append(pm[:, c0:512])
                    srcs_q[kt] = (srcs, c0)

                def emit_av(kt):
                    srcs, c0 = srcs_q.pop(kt)
                    for h in range(HP):
                        nc.tensor.matmul(
                            av[h][0:65, c0:512],
                            v_sb[:, b * KT_PER_S + kt, h, 0:65],
                            srcs[h],
                            start=(kt == 0),
                            stop=(kt == nkt - 1),
                        )

                for kt in range(LEAD):
                    emit_scores(kt)
                for kt in range(nkt):
                    if kt + LEAD < nkt:
                        emit_scores(kt + LEAD)
                    emit_av(kt)
                    drain(1)
                # epilogue part 1 (prompt): free the av PSUM tiles fast — raw
                # av rows straight to SBUF (bf16) on DVE, denominator row on
                # ScalarE — and kick off the denominator's DRAM round-trip
                # (reshapes [1,512] -> [128,4] so the DVE reciprocal costs
                # 0.19us instead of 3.3us; 8 cyc per FREE element, serial).
                # The reciprocal itself is DEFERRED one chunk (via the filler
                # queue) so its DMA latency never stalls the strict-FIFO
                # DVE queue.
                if attnT is None:
                    attnT = misc.tile([128, 512], BF16, tag="attnT", bufs=6)
                den4s = []
                for h in range(HP):
                    den = misc.tile([1, 512], F32, tag="den", bufs=4)
                    nc.scalar.copy(den[:], av[h][64:65, :])
                    nc.vector.tensor_copy(
                        attnT[64 * h : 64 * (h + 1), :], av[h][0:64, :]
                    )
                    nc.gpsimd.dma_start(den_dram[ch, h, :, :], den[:])
                    den4 = misc.tile([128, 4], F32, tag="den4", bufs=6)
                    nc.gpsimd.dma_start(
                        den4[:],
                        den_dram[ch, h, 0, :].rearrange("(p f) -> p f", p=128),
                    )
                    den4s.append(den4)
                return (b, qt, ch, attnT, den4s)

            def finish_a(state):
                # epilogue part 2 (deferred by one chunk): reciprocal and the
                # partition-broadcast DMA via DRAM. den4 arrived long ago, so
                # the DVE reciprocal never stalls the FIFO.
                b, qt, ch, attnT, den4s = state
                rb = misc.tile([128, 512], BF16, tag="rb", bufs=3)
                for h in range(HP):
                    rec4 = misc.tile([128, 4], BF16, tag="rec4", bufs=4)
                    with nc.allow_low_precision(
                        reason="bf16 recip: 0.4% rel err, tolerance is 2e-2"
                    ):
                        nc.vector.reciprocal(rec4[:], den4s[h][:])
                    nc.gpsimd.dma_start(
                        rec_dram[ch, h, 0, :].rearrange("(p f) -> p f", p=128),
                        rec4[:],
                    )
                    nc.gpsimd.dma_start(
                        rb[64 * h : 64 * (h + 1), :],
                        rec_dram[ch, h, :, :].to_broadcast((64, 512)),
                    )
                return state + (rb,)

            def finish_mult(state):
                # epilogue part 3: one full-width in-place normalize multiply
                # (bf16 SBUF x SBUF, 2x DVE mode).
                b, qt, ch, attnT, den4s, rb = state
                nc.vector.tensor_tensor(
                    attnT[:], attnT[:], rb[:], mybir.AluOpType.mult
                )

            def outproj_half(state, which):
                # partial output projection (half of a chunk's 512 tokens):
                # out[tok, :] += attnT[:, tok].T @ Wo[128 core rows, :]
                b, qt = state[0], state[1]
                attnT = state[3]
                row0 = 2048 * b + 512 * qt
                for tt in (2 * which, 2 * which + 1):
                    for half in range(2):
                        pso = ps512.tile(
                            [128, 512],
                            F32,
                            tag="ps512",
                            name=f"o{b}_{qt}_{tt}_{half}",
                        )
                        nc.tensor.matmul(
                            pso[:],
                            attnT[:, 128 * tt : 128 * (tt + 1)],
                            wo_sb[:, 512 * half : 512 * (half + 1)],
                            start=True,
                            stop=True,
                        )
                        osb = stage.tile([128, 512], BF16, tag="osb", bufs=6)
                        nc.vector.tensor_copy(osb[:], pso[:])
                        nc.sync.dma_start(
                            out[
                                row0 + 128 * tt : row0 + 128 * (tt + 1),
                                512 * half : 512 * (half + 1),
                            ],
                            osb[:],
                        )

            # ---- emission: chunks 0/1 of qkv inline, then the 16 attention
            # chunks. Filler pieces are drained one per attention k-tile.
            # Per-chunk pipeline depth: finish_a (reciprocal+broadcast DMAs)
            # drains one chunk after the attention; finish_mult + output
            # projection drain TWO chunks after — so every DVE filler op's
            # DMA inputs have landed long before the strict-FIFO DVE queue
            # reaches it. Two early output projections are RESERVED as
            # pure-PE tail filler (dedicated attnT tiles so the ring never
            # waits on them).
            reserved = []
            for p in qkv_pieces(0, strip=True):
                p()
            if n_mask_tiles:
                nc.sync.dma_start(mt_sb[:], mt.rearrange("m p q -> p m q"))
            nc.sync.dma_start(wo_sb[:], wo[:, :])
            # qkv(1) rides the filler queue: attention(0,0) only needs chunk
            # 0, so its k-tiles interleave qkv(1)'s matmuls while the x1 DMA
            # is still streaming in.
            fillers.extend(qkv_pieces(1, strip=True))
            order = [(b, qt) for b in range(B) for qt in range(QT_PER_S)]
            boxes = {}
            for idx, (b, qt) in enumerate(order):
                reserve = idx < 3
                ded = (
                    const.tile([128, 512], BF16, name=f"attnT_r{idx}")
                    if reserve
                    else None
                )
                s = attention(b, qt, ded)
                box = {"s": s, "reserve": reserve}
                boxes[idx] = box

                if idx + 2 < NCH:
                    fillers.extend(qkv_pieces(idx + 2))
                if idx - 1 in boxes:
                    pb = boxes[idx - 1]
                    fillers.append(
                        lambda pb=pb: finish_mult(pb["st"])
                    )
                    ops = [
                        (lambda pb=pb: outproj_half(pb["st"], 0)),
                        (lambda pb=pb: outproj_half(pb["st"], 1)),
                    ]
                    if pb["reserve"]:
                        reserved.extend(ops)
                    else:
                        fillers.extend(ops)
                fillers.append(
                    lambda box=box: box.__setitem__(
                        "st", finish_a(box["s"])
                    )
                )
            # tail: finish the last two chunks, lacing the reserved
            # (pure-PE) output projections between the DMA-latency-bound
            # pieces so the PE stays busy while the reciprocal chains fly.
            tail = list(fillers)
            fillers.clear()
            for piece in tail:
                piece()
            pb = boxes[len(order) - 1]
            for op in reserved:
                op()
            finish_mult(pb["st"])
            outproj_half(pb["st"], 0)
            outproj_half(pb["st"], 1)
    _split_waits(nc)

    # The libneuronxla NEFF cache hashes the HLO, but the BIR travels in
    # backend_config which is NOT part of the hash — two different kernels
    # with identical I/O signatures collide and the stale NEFF runs. Encode
    # a hash of the BIR into the shape of an unused dummy input so the HLO
    # (and therefore the cache key) changes whenever the kernel changes.
    import hashlib

    hv = int.from_bytes(
        hashlib.sha256(nc.to_json_bytes()).digest()[:4], "little"
    )
    nonce_shape = [hv % 1021 + 1, (hv // 1021) % 1021 + 1]
    nc.dram_tensor("nonce", nonce_shape, F32, kind="ExternalInput")
    nc._nonce_shape = nonce_shape
    return nc


_BUILD_CACHE = {}


def _get_nc(mode, n_mask_tiles):
    key = (mode, n_mask_tiles)
    if key not in _BUILD_CACHE:
        _BUILD_CACHE[key] = build(mode, n_mask_tiles)
    return _BUILD_CACHE[key]


def kernel(x, Wqkv, Wo, mask):
    x = np.asarray(x)
    Wqkv = np.asarray(Wqkv)
    Wo = np.asarray(Wo)
    mask = np.asarray(mask)

    m2 = mask.reshape(S, S)
    if np.array_equal(m2, np.tril(np.ones((S, S), bool))):
        mode = "causal"
    elif m2.all():
        mode = "full"
    else:
        mode = "general"

    BF = ml_dtypes.bfloat16
    # host-side input prep: transpose+cast x, slice per-head weight shards
    xT = np.ascontiguousarray(x.reshape(T, DM).T.astype(BF))
    w4 = Wqkv.reshape(DM, H, 3, D)

    if mode == "causal":
        # mask tile for diagonal offset o: [k=128, q=512], 1 where q >= k + 128*o
        qq = np.arange(512)[None, :]
        kk = np.arange(128)[:, None]
        mts = np.stack(
            [(qq - kk >= 128 * o) for o in range(4)]
        ).astype(BF)
        n_mask_tiles = 4
    elif mode == "general":
        tiles = []
        for qt in range(QT_PER_S):
            for kt in range(KT_PER_S):
                sub = m2[512 * qt : 512 * (qt + 1), 128 * kt : 128 * (kt + 1)]
                tiles.append(sub.T)
        mts = np.stack(tiles).astype(BF)
        n_mask_tiles = len(tiles)
    else:
        mts = None
        n_mask_tiles = 0

    nc = _get_nc(mode, n_mask_tiles)

    in_maps = []
    for j in range(NCORES):
        hs = slice(HP * j, HP * (j + 1))
        im = {
            "xT": xT,
            "wq": np.ascontiguousarray(
                w4[:, hs, 0, :].reshape(DM, HP * D).astype(BF)
            ),
            "wk": np.ascontiguousarray(
                w4[:, hs, 1, :].reshape(DM, HP * D).astype(BF)
            ),
            "wv": np.ascontiguousarray(
                w4[:, hs, 2, :].reshape(DM, HP * D).astype(BF)
            ),
            "wo": np.ascontiguousarray(
                Wo[128 * j : 128 * (j + 1), :].astype(BF)
            ),
            "nonce": np.zeros(nc._nonce_shape, np.float32),
        }
        if n_mask_tiles:
            im["mt"] = mts
        in_maps.append(im)

    res = run_bass_kernel_spmd(nc, in_maps, list(range(NCORES)))
    # host all-reduce of the 8 partial projections
    acc = np.zeros((T, DM), np.float32)
    for j in range(NCORES):
        acc += res.results[j]["out"].astype(np.float32)
    return acc.reshape(B, S, DM)


if __name__ == "__main__":
    rng = np.random.default_rng(0)
    x = rng.standard_normal((B, S, DM), dtype=np.float32)
    Wqkv = rng.standard_normal((DM, 3 * H * D), dtype=np.float32) * DM**-0.5
    Wo = rng.standard_normal((H * D, DM), dtype=np.float32) * (H * D) ** -0.5
    mask = np.tril(np.ones((S, S), bool))[None, None]
    out = kernel(x=x, Wqkv=Wqkv, Wo=Wo, mask=mask)
    print(out.shape, out.dtype)
